# revision 36
# baseline (speedup 1.0000x reference)
"""Trainium2 Bass kernel for BatchedStarNetAttentionBlock.

Strategy: data-parallel over ordering segments (attention is block-diagonal,
never crosses segment boundaries). Each of the 8 cores gets a subset of
segments, padded to a shared static structure so one SPMD program serves all
cores. No collectives.

On-device layout: activations are kept transposed, xT[d, n] with the feature
dim on partitions (2 tiles of 128), so every linear layer is a natural
matmul (lhsT = weight chunk [k,j], rhs = xT chunk [k,n]). Scores are computed
directly in transposed form S.T = kT.T @ qT (keys on partitions), so softmax
exp output P.T feeds the PV matmul without any transpose. PV is col-tiled by
head (tile_position=(0,32h)) so attention output lands as oT[d, n] in PSUM.
Denominators come from ones-matmuls writing a row-replicated bank with the
same layout as oT, so normalization is a single fused multiply+copy.
"""

import sys

for _p in ("/opt/trn_rl_repo",):
    if _p not in sys.path:
        sys.path.insert(0, _p)

import numpy as np
import ml_dtypes

import bass_rust as _bass_rust

import concourse.bass as bass
import concourse.tile as tile
from concourse import bacc
from concourse import mybir
from concourse.bass_utils import run_bass_kernel_spmd
from concourse.hw_specs import get_activation_tables


class _Bacc(bacc.Bacc):
    """Bacc whose activation-table planner prefers the set that contains
    exp+ln+square+identity together, so per-layernorm Ln/Exp pairs do not
    ping-pong ACT table loads (~2.6us per switch)."""

    def insert_act_table_loads(self):
        has_activation = any(
            isinstance(i, mybir.InstActivation)
            for b in self.main_func.blocks
            for i in b.instructions
        )
        if not has_activation:
            return
        tables = list(get_activation_tables(self.m.arch).items())
        # The planner emits act_func_set_id = position in this list, so
        # positions must stay aligned with act_info.json. Narrow the match
        # sets instead: position 0 claims only tanh; other sets before
        # natural_log_exp_and_others claim nothing; so exp/ln/square/
        # identity/copy all resolve to the one set that has them all.
        pref = "natural_log_exp_and_others"
        TANH = mybir.ActivationFunctionType.Tanh
        doctored = []
        seen_pref = False
        for name, fns in tables:
            if name == pref:
                seen_pref = True
                doctored.append((name, fns))
            elif not seen_pref:
                doctored.append((name, {TANH} & fns))
            else:
                doctored.append((name, fns))
        _bass_rust.insert_act_table_loads(self, doctored)

P = 128
D = 256
H = 8
DH = 32
SCALE = 1.0 / float(np.sqrt(DH))
N_CORES = 8
NEG = -1e9

F32 = mybir.dt.float32
BF16 = mybir.dt.bfloat16

# activation dtype switch ("f32" or "bf16")
DT_ACT_NAME = "bf16"


def _dt_act():
    return BF16 if DT_ACT_NAME == "bf16" else F32


def _np_act():
    return ml_dtypes.bfloat16 if DT_ACT_NAME == "bf16" else np.float32


# ---------------------------------------------------------------------------
# weight packing layout (shared between host packer and device program)
# ---------------------------------------------------------------------------
# W_all [128, n_wcols] (dt_act): matmul weight chunks, 128 cols each.
#   chunk_col(base, k, j) = base + k*(2*128) + j*128   (k-outer, j-inner)
#   lin_W  at base 0                      (4 chunks)
#   Wq[i]  at 512 + i*2048 + 0
#   Wk[i]  at 512 + i*2048 + 512
#   Wv[i]  at 512 + i*2048 + 1024
#   Wo[i]  at 512 + i*2048 + 1536
N_WCOLS = 512 + 2 * 2048

LIN_BASE = 0


def w_base(i, which):
    return 512 + i * 2048 + {"q": 0, "k": 512, "v": 1024, "o": 1536}[which]


# C_all [128, n_ccols] f32: per-feature columns (partition = feature within
# d-tile j). col index helpers:
#   0,1   lin_b (j=0,1)
#   2,3   lin_g
#   4,5   lin_beta
#   6+i*12 + [0,1]=bq, [2,3]=bk, [4,5]=bv, [6,7]=bo, [8,9]=ln_g, [10,11]=ln_b
#   30..30+T  maskbias columns (per key-tile)
def c_lin(which, j):
    return {"b": 0, "g": 2, "beta": 4}[which] + j


def c_blk(i, which, j):
    return 6 + i * 12 + {"q": 0, "k": 2, "v": 4, "o": 6, "g": 8, "beta": 10}[which] + j


C_FIXED = 30


# ---------------------------------------------------------------------------
# device program
# ---------------------------------------------------------------------------
def build_program(slot_ts, trivial_ln):
    """slot_ts: tuple of per-slot tile counts (shared across cores).
    trivial_ln: all LN gains are 1 and shifts 0 (skip gamma/beta application)
    """
    dt = _dt_act()
    T = int(sum(slot_ts))
    NC = T * P  # padded node count per core
    CHW = 512  # chunk width for the n dimension
    NCH = [(c0, min(CHW, NC - c0)) for c0 in range(0, NC, CHW)]  # n chunks

    nc = _Bacc()
    featT = nc.declare_dram_parameter("featT", [P, 2, NC], dt, isOutput=False)
    wall = nc.declare_dram_parameter("wall", [P, N_WCOLS], dt, isOutput=False)
    cons = nc.declare_dram_parameter("cons", [P, C_FIXED + T], F32, isOutput=False)
    outT = nc.declare_dram_parameter("outT", [P, 2, NC], F32, isOutput=True)

    with tile.TileContext(nc) as tc:
        with (
            tc.tile_pool(name="wp", bufs=1) as wp,
            tc.tile_pool(name="xp", bufs=1) as xp,
            tc.tile_pool(name="pp", bufs=max(4, 2 * max(slot_ts))) as pp,
            tc.tile_pool(name="rows", bufs=2) as rows,
            tc.tile_pool(name="psA", bufs=4, space="PSUM") as psA,
            tc.tile_pool(name="psB", bufs=1, space="PSUM") as psB,
        ):
            # separate weight tiles per stage so consumers only wait on
            # their own DMA (tile-granular deps)
            w_lin = wp.tile([P, 512], dt, tag="w_lin")
            w_blk = [wp.tile([P, 2048], dt, tag=f"w_blk{i}", name=f"w_blk{i}")
                     for i in range(2)]
            c_sb = wp.tile([P, C_FIXED + T], F32, tag="c")
            nc.sync.dma_start(c_sb[:], cons[:])
            nc.sync.dma_start(w_lin[:], wall[:, 0:512])
            # big weight loads go on other queues so x0/w_lin aren't stuck
            # behind them and block0 can start immediately
            nc.scalar.dma_start(w_blk[0][:], wall[:, 512:2560])
            nc.gpsimd.dma_start(w_blk[1][:], wall[:, 2560:4608])

            def w_tile_of(base):
                if base < 512:
                    return w_lin, base
                i = (base - 512) // 2048
                return w_blk[i], (base - 512) % 2048

            x0 = [xp.tile([P, NC], dt, tag=f"x0_{k}", name=f"x0_{k}") for k in range(2)]
            nc.sync.dma_start(x0[0][:], featT[:, 0, :])
            nc.sync.dma_start(x0[1][:], featT[:, 1, :])

            # constants
            ones32 = wp.tile([P, 32], BF16, tag="ones32")
            nc.vector.memset(ones32, 1.0)
            c256 = wp.tile([P, 1], dt, tag="c256")
            nc.vector.memset(c256, 1.0 / 256.0)
            ones_row = wp.tile([1, P], dt, tag="ones_row")
            nc.vector.memset(ones_row, 1.0)
            eps_row = wp.tile([1, 1], F32, tag="eps_row")
            nc.vector.memset(eps_row, 1e-5)

            def wcol(base, k, j, width=P):
                wt, rel = w_tile_of(base)
                c0 = rel + k * 256 + j * 128
                return wt[:, c0 : c0 + width]

            def ccol(idx):
                return c_sb[:, idx : idx + 1]

            def r32(ap):
                # float32r: same bits as f32, single-pass PE mode (vs the
                # 2-pass LOW_HIGH fp32 lowering)
                return ap.bitcast(mybir.dt.float32r) if ap.dtype == F32 else ap

            def linearT(src, base, bias_idx, func, out_dt, eng="act"):
                """out[j][d,n] = func(sum_k W[k,j].T @ src[k] + bias_j)"""
                out = [xp.tile([P, NC], out_dt, tag=f"lt{base}_{j}", name=f"lt{base}_{j}") for j in range(2)]
                for j in range(2):
                    for c0, cw in NCH:
                        ps = psA.tile([P, cw], F32, tag="work")
                        for k in range(2):
                            nc.tensor.matmul(
                                ps,
                                r32(wcol(base, k, j)),
                                r32(src[k][:, c0 : c0 + cw]),
                                start=(k == 0),
                                stop=(k == 1),
                            )
                        if eng == "dve":
                            nc.vector.tensor_scalar_add(
                                out[j][:, c0 : c0 + cw], ps, ccol(bias_idx + j)
                            )
                        else:
                            nc.scalar.activation(
                                out[j][:, c0 : c0 + cw], ps, func,
                                bias=ccol(bias_idx + j),
                            )
                return out

            def layernormT(y, gcol, bcol, out_dt, trivial):
                """LayerNorm over feature dim (partitions, 2 tiles of 128)."""
                out = [xp.tile([P, NC], out_dt, tag=f"ln_{k}", name=f"ln_{k}") for k in range(2)]
                # squared input (same dtype for matmul rhs)
                sq = [xp.tile([P, NC], dt, tag=f"sq_{k}", name=f"sq_{k}") for k in range(2)]
                nc.vector.tensor_mul(sq[1], y[1], y[1])
                nc.gpsimd.tensor_mul(sq[0], y[0], y[0])
                for c0, cw in NCH:
                    stats = psA.tile([33, cw], F32, tag="work")
                    for k in range(2):
                        nc.tensor.matmul(
                            stats[0:1, :],
                            r32(c256),
                            r32(y[k][:, c0 : c0 + cw]),
                            start=(k == 0),
                            stop=(k == 1),
                        )
                    for k in range(2):
                        nc.tensor.matmul(
                            stats[32:33, :],
                            r32(c256),
                            r32(sq[k][:, c0 : c0 + cw]),
                            start=(k == 0),
                            stop=(k == 1),
                            tile_position=(0, 32),
                        )
                    # rows: var = E[y^2] - mean^2 ; rstd = 1/sqrt(var+eps)
                    m2 = rows.tile([1, cw], F32, tag="m2")
                    nc.scalar.activation(
                        m2, stats[0:1, :], mybir.ActivationFunctionType.Square
                    )
                    var = rows.tile([1, cw], F32, tag="var")
                    nc.vector.scalar_tensor_tensor(
                        var,
                        m2,
                        -1.0,
                        stats[32:33, :],
                        op0=mybir.AluOpType.mult,
                        op1=mybir.AluOpType.add,
                    )
                    # rstd = exp(-0.5*ln(var+eps)) — Ln/Exp share one ACT
                    # table set; Sqrt would force a ~2.7us table switch
                    lnv = rows.tile([1, cw], F32, tag="lnv")
                    nc.scalar.activation(
                        lnv, var, mybir.ActivationFunctionType.Ln, bias=eps_row[:]
                    )
                    rstd = rows.tile([1, cw], dt, tag="rstd")
                    nc.scalar.activation(
                        rstd, lnv, mybir.ActivationFunctionType.Exp, scale=-0.5
                    )
                    ms = rows.tile([1, cw], dt, tag="ms")
                    nc.vector.tensor_mul(ms, stats[0:1, :], rstd)
                    # broadcast rows to 128 partitions via K=1 matmuls
                    sb = psA.tile([P, cw], F32, tag="work")
                    nc.tensor.matmul(sb, r32(ones_row), r32(rstd), start=True, stop=True)
                    msb = psA.tile([P, cw], F32, tag="work")
                    nc.tensor.matmul(msb, r32(ones_row), r32(ms), start=True, stop=True)
                    for k in range(2):
                        t1 = xp.tile([P, cw], F32, tag="ln_t1")
                        nc.vector.tensor_mul(t1, y[k][:, c0 : c0 + cw], sb)
                        dst = out[k][:, c0 : c0 + cw]
                        if trivial:
                            nc.vector.tensor_sub(dst, t1, msb)
                        else:
                            t2 = xp.tile([P, cw], F32, tag="ln_t2")
                            nc.vector.tensor_sub(t2, t1, msb)
                            nc.vector.tensor_scalar(
                                dst,
                                t2,
                                ccol(gcol + k),
                                ccol(bcol + k),
                                op0=mybir.AluOpType.mult,
                                op1=mybir.AluOpType.add,
                            )
                return out

            def attention_block(i, hT):
                """one MHA block: returns new xT (list of 2 tiles)."""
                qT = linearT(
                    hT, w_base(i, "q"), c_blk(i, "q", 0),
                    mybir.ActivationFunctionType.Identity, BF16,
                )
                kT = linearT(
                    hT, w_base(i, "k"), c_blk(i, "k", 0),
                    mybir.ActivationFunctionType.Identity, BF16, eng="dve",
                )
                # matmul operands must start at partition 0 on this stack, so
                # shift each head's 32 rows down via SBUF->SBUF DMA
                q_h = xp.tile([32, H, NC], BF16, tag="q_h", name="q_h")
                k_h = xp.tile([32, H, NC], BF16, tag="k_h", name="k_h")
                # spread the 16 shift-DMAs across engine queues so they run
                # in parallel instead of serializing on the sync queue
                engs = [nc.sync, nc.gpsimd, nc.scalar]
                for h in range(H):
                    b, hh = divmod(h, 4)
                    engs[h % 3].dma_start(
                        q_h[:, h, :], qT[b][32 * hh : 32 * hh + 32, :]
                    )
                    engs[(h + 1) % 3].dma_start(
                        k_h[:, h, :], kT[b][32 * hh : 32 * hh + 32, :]
                    )
                # v in node layout: v[n_tile, d] = hT_chunk.T @ Wv_chunk
                v_sb = xp.tile([P, T, 256], BF16, tag="v_all")
                for t in range(T):
                    vp = psA.tile([P, 256], F32, tag="work")
                    for k in range(2):
                        nc.tensor.matmul(
                            vp,
                            r32(hT[k][:, t * P : (t + 1) * P]),
                            r32(wcol(w_base(i, "v"), k, 0, width=256)),
                            start=(k == 0),
                            stop=(k == 1),
                        )
                    nc.scalar.activation(
                        v_sb[:, t, :], vp, mybir.ActivationFunctionType.Copy
                    )
                # attention per slot
                o_sb = [xp.tile([P, NC], dt, tag=f"o_{k}", name=f"o_{k}") for k in range(2)]
                # oT/DN accumulate banks per (dtile, nchunk)
                all_unit = all(t == 1 for t in slot_ts)
                for c0, cw in NCH:
                    oT_ps = [psB.tile([P, cw], F32, tag=f"oT{k}", name=f"oT{k}") for k in range(2)]
                    dn_ps = None
                    if not all_unit:
                        dn_ps = [psB.tile([P, cw], F32, tag=f"dn{k}", name=f"dn{k}") for k in range(2)]
                    # pT for the whole chunk lives in one tile per bank so
                    # the denominator can be a single N=cw matmul per head
                    ctiles = cw // P
                    pT_all = [
                        pp.tile([P, ctiles, 4, P], BF16, tag=f"pTall{b}",
                                name=f"pTall{b}")
                        for b in range(2)
                    ]
                    t_off = 0  # key-tile offset (global)
                    q_off = 0
                    for s, ts_s in enumerate(slot_ts):
                        for qc0 in range(q_off, q_off + ts_s * P, P):
                            if qc0 < c0 or qc0 >= c0 + cw:
                                continue
                            qrel = qc0 - c0
                            qi = qrel // P
                            # scores + exp for all key tiles of this slot
                            pT = {}
                            for kt in range(ts_s):
                                ktg = t_off + kt
                                for b in range(2):
                                    stp = psA.tile([P, 4, P], F32, tag="work")
                                    for hh in range(4):
                                        nc.tensor.matmul(
                                            stp[:, hh, :],
                                            k_h[:, 4 * b + hh,
                                                ktg * P : (ktg + 1) * P],
                                            q_h[:, 4 * b + hh, qc0 : qc0 + P],
                                            start=True,
                                            stop=True,
                                        )
                                    if all_unit:
                                        p_t = pT_all[b][:, qi, :, :]
                                    else:
                                        p_t = pp.tile([P, 4, P], BF16, tag="pT",
                                                      name="p_t")
                                    nc.scalar.activation(
                                        p_t,
                                        stp,
                                        mybir.ActivationFunctionType.Exp,
                                        scale=SCALE,
                                        bias=ccol(C_FIXED + ktg),
                                    )
                                    pT[kt, b] = p_t
                            # PV accumulation, one closed psum group per
                            # (bank, head) at a time
                            for b in range(2):
                                for hh in range(4):
                                    for kt in range(ts_s):
                                        nc.tensor.matmul(
                                            oT_ps[b][32 * hh : 32 * hh + 32,
                                                     qrel : qrel + P],
                                            v_sb[:, t_off + kt,
                                                 (4 * b + hh) * 32 : (4 * b + hh) * 32 + 32],
                                            pT[kt, b][:, hh, :],
                                            start=(kt == 0),
                                            stop=(kt == ts_s - 1),
                                            tile_position=(0, 32 * hh),
                                        )
                                    if not all_unit:
                                        for kt in range(ts_s):
                                            nc.tensor.matmul(
                                                dn_ps[b][32 * hh : 32 * hh + 32,
                                                         qrel : qrel + P],
                                                ones32,
                                                pT[kt, b][:, hh, :],
                                                start=(kt == 0),
                                                stop=(kt == ts_s - 1),
                                                tile_position=(0, 32 * hh),
                                            )
                        t_off += ts_s
                        q_off += ts_s * P
                    if all_unit:
                        # merged denominators: each slot's keys live on the
                        # partition axis of its own pT block, so one matmul
                        # per (bank, head) column-sums the whole chunk
                        dn_ps = [psB.tile([P, cw], F32, tag=f"dn{k}", name=f"dn{k}")
                                 for k in range(2)]
                        for b in range(2):
                            for hh in range(4):
                                nc.tensor.matmul(
                                    dn_ps[b][32 * hh : 32 * hh + 32, :],
                                    ones32,
                                    pT_all[b][:, :, hh, :],
                                    start=True,
                                    stop=True,
                                    tile_position=(0, 32 * hh),
                                )
                    # normalize: o = oT * (1/dn), fused with PSUM->SBUF copy
                    for k in range(2):
                        r_sb = pp.tile([P, cw], F32, tag="r")
                        nc.vector.reciprocal_approx_fast(out=r_sb, in_=dn_ps[k])
                        nc.vector.tensor_mul(o_sb[k][:, c0 : c0 + cw], oT_ps[k], r_sb)
                # output projection + residual + LN
                y = [xp.tile([P, NC], dt, tag=f"y_{k}", name=f"y_{k}") for k in range(2)]
                for j in range(2):
                    for c0, cw in NCH:
                        zp = psA.tile([P, cw], F32, tag="work")
                        for k in range(2):
                            nc.tensor.matmul(
                                zp,
                                r32(wcol(w_base(i, "o"), k, j)),
                                r32(o_sb[k][:, c0 : c0 + cw]),
                                start=(k == 0),
                                stop=(k == 1),
                            )
                        nc.vector.scalar_tensor_tensor(
                            y[j][:, c0 : c0 + cw],
                            zp,
                            ccol(c_blk(i, "o", j)),
                            hT[j][:, c0 : c0 + cw],
                            op0=mybir.AluOpType.add,
                            op1=mybir.AluOpType.add,
                        )
                out_dt = F32 if i == 1 else dt
                return layernormT(
                    y, c_blk(i, "g", 0), c_blk(i, "beta", 0), out_dt, trivial_ln
                )

            # block 0 pre-layer: LN(tanh(x @ lin_W + lin_b)) * g + beta
            t0 = linearT(
                x0, LIN_BASE, c_lin("b", 0), mybir.ActivationFunctionType.Tanh, dt
            )
            h0 = layernormT(t0, c_lin("g", 0), c_lin("beta", 0), dt, trivial_ln)
            x1 = attention_block(0, h0)
            x2 = attention_block(1, x1)
            nc.sync.dma_start(outT[:, 0, :], x2[0][:])
            nc.sync.dma_start(outT[:, 1, :], x2[1][:])

    nc.finalize()
    return nc


# ---------------------------------------------------------------------------
# host side
# ---------------------------------------------------------------------------
_prog_cache = {}
_last_results = None


def _get_program(slot_ts, trivial_ln):
    key = (tuple(slot_ts), trivial_ln, DT_ACT_NAME)
    if key not in _prog_cache:
        _prog_cache[key] = build_program(tuple(slot_ts), trivial_ln)
    return _prog_cache[key]


def _segments(ordering):
    """contiguous runs of equal values in sorted ordering -> (start, len)."""
    n = ordering.shape[0]
    change = np.nonzero(np.diff(ordering))[0] + 1
    starts = np.concatenate([[0], change])
    lens = np.diff(np.concatenate([starts, [n]]))
    return list(zip(starts.tolist(), lens.tolist()))


def kernel(
    feat, ordering, lin_W, lin_b, lin_g, lin_beta,
    Wq, Wk, Wv, bq, bk, bv, Wo, bo, ln_g, ln_b,
):
    feat = np.asarray(feat, np.float32)
    ordering = np.asarray(ordering)
    N = feat.shape[0]
    np_act = _np_act()

    perm = np.argsort(ordering, kind="stable")
    segs = _segments(np.asarray(ordering)[perm])

    # deal segments (sorted by length desc) snake-wise to cores
    order = sorted(range(len(segs)), key=lambda i: -segs[i][1])
    core_slots = [[] for _ in range(N_CORES)]
    for r, si in enumerate(order):
        c = r % (2 * N_CORES)
        c = c if c < N_CORES else 2 * N_CORES - 1 - c
        core_slots[c].append(si)
    S = max(len(cs) for cs in core_slots)
    # per-slot tile count = max over cores of the slot's segment size
    slot_ts = []
    for k in range(S):
        mx = 1
        for c in range(N_CORES):
            if k < len(core_slots[c]):
                mx = max(mx, (segs[core_slots[c][k]][1] + P - 1) // P)
        slot_ts.append(mx)
    T = sum(slot_ts)
    NC = T * P

    trivial_ln = bool(
        np.all(np.asarray(ln_g) == 1) and np.all(np.asarray(ln_b) == 0)
        and np.all(np.asarray(lin_g) == 1) and np.all(np.asarray(lin_beta) == 0)
    )
    # softmax rows sum to 1, so attn @ (v + 1*bv^T) = attn@v + 1*bv^T; the
    # per-block value bias folds exactly into the output-projection bias:
    # bo_eff = bo + bv @ Wo
    bo = np.asarray(bo, np.float32) + np.einsum(
        "id,idj->ij", np.asarray(bv, np.float32), np.asarray(Wo, np.float32)
    )

    nc = _get_program(slot_ts, trivial_ln)

    # ---- pack weights ----
    wallp = np.zeros((P, N_WCOLS), np.float32)

    def put_w(base, W):
        W = np.asarray(W, np.float32)
        for k in range(2):
            for j in range(2):
                c0 = base + k * 256 + j * 128
                wallp[:, c0 : c0 + 128] = W[k * 128 : (k + 1) * 128,
                                            j * 128 : (j + 1) * 128]

    put_w(LIN_BASE, lin_W)
    for i in range(2):
        put_w(w_base(i, "q"), np.asarray(Wq)[i])
        put_w(w_base(i, "k"), np.asarray(Wk)[i])
        put_w(w_base(i, "v"), np.asarray(Wv)[i])
        put_w(w_base(i, "o"), np.asarray(Wo)[i])
    wallp = wallp.astype(np_act)

    consp = np.zeros((P, C_FIXED + T), np.float32)

    def put_c(idx, vec):
        vec = np.asarray(vec, np.float32)
        consp[:, idx] = vec[:128]
        consp[:, idx + 1] = vec[128:]

    put_c(c_lin("b", 0), lin_b)
    put_c(c_lin("g", 0), lin_g)
    put_c(c_lin("beta", 0), lin_beta)
    for i in range(2):
        put_c(c_blk(i, "q", 0), np.asarray(bq)[i])
        put_c(c_blk(i, "k", 0), np.asarray(bk)[i])
        put_c(c_blk(i, "v", 0), np.asarray(bv)[i] * 0)
        put_c(c_blk(i, "o", 0), np.asarray(bo)[i])
        put_c(c_blk(i, "g", 0), np.asarray(ln_g)[i])
        put_c(c_blk(i, "beta", 0), np.asarray(ln_b)[i])

    # ---- per-core data ----
    feat_sorted = feat[perm]
    in_maps = []
    core_meta = []  # (slot k, seg_start, seg_len, node_offset in padded layout)
    for c in range(N_CORES):
        fT = np.zeros((NC, 256), np.float32)
        mb = np.full((NC,), NEG, np.float32)
        meta = []
        off = 0
        for k in range(S):
            if k < len(core_slots[c]):
                st, ln = segs[core_slots[c][k]]
                fT[off : off + ln] = feat_sorted[st : st + ln]
                mb[off : off + ln] = 0.0
                meta.append((st, ln, off))
            off += slot_ts[k] * P
        cons_c = consp.copy()
        cons_c[:, C_FIXED : C_FIXED + T] = mb.reshape(T, P).T
        featT_c = np.ascontiguousarray(
            fT.T.reshape(2, P, NC).transpose(1, 0, 2)
        ).astype(np_act)
        in_maps.append({"featT": featT_c, "wall": wallp, "cons": cons_c})
        core_meta.append(meta)

    res = run_bass_kernel_spmd(nc, in_maps, list(range(N_CORES)))
    global _last_results
    _last_results = res

    out = np.empty((N, 256), np.float32)
    for c in range(N_CORES):
        oT = np.asarray(res.results[c]["outT"], np.float32)  # [128, 2, NC]
        o_nodes = oT.transpose(1, 0, 2).reshape(256, NC).T  # [NC, 256]
        for st, ln, off in core_meta[c]:
            out[perm[st : st + ln]] = o_nodes[off : off + ln]
    return out


# revision 39
# speedup vs baseline: 1.0053x; 1.0053x over previous
"""Trainium2 Bass kernel for BatchedStarNetAttentionBlock.

Strategy: data-parallel over ordering segments (attention is block-diagonal,
never crosses segment boundaries). Each of the 8 cores gets a subset of
segments, padded to a shared static structure so one SPMD program serves all
cores. No collectives.

On-device layout: activations are kept transposed, xT[d, n] with the feature
dim on partitions (2 tiles of 128), so every linear layer is a natural
matmul (lhsT = weight chunk [k,j], rhs = xT chunk [k,n]). Scores are computed
directly in transposed form S.T = kT.T @ qT (keys on partitions), so softmax
exp output P.T feeds the PV matmul without any transpose. PV is col-tiled by
head (tile_position=(0,32h)) so attention output lands as oT[d, n] in PSUM.
Denominators come from ones-matmuls writing a row-replicated bank with the
same layout as oT, so normalization is a single fused multiply+copy.
"""

import sys

for _p in ("/opt/trn_rl_repo",):
    if _p not in sys.path:
        sys.path.insert(0, _p)

import numpy as np
import ml_dtypes

import bass_rust as _bass_rust

import concourse.bass as bass
import concourse.tile as tile
from concourse import bacc
from concourse import mybir
from concourse.bass_utils import run_bass_kernel_spmd
from concourse.hw_specs import get_activation_tables


class _Bacc(bacc.Bacc):
    """Bacc whose activation-table planner prefers the set that contains
    exp+ln+square+identity together, so per-layernorm Ln/Exp pairs do not
    ping-pong ACT table loads (~2.6us per switch)."""

    def insert_act_table_loads(self):
        has_activation = any(
            isinstance(i, mybir.InstActivation)
            for b in self.main_func.blocks
            for i in b.instructions
        )
        if not has_activation:
            return
        tables = list(get_activation_tables(self.m.arch).items())
        # The planner emits act_func_set_id = position in this list, so
        # positions must stay aligned with act_info.json. Narrow the match
        # sets instead: position 0 claims only tanh; other sets before
        # natural_log_exp_and_others claim nothing; so exp/ln/square/
        # identity/copy all resolve to the one set that has them all.
        pref = "natural_log_exp_and_others"
        TANH = mybir.ActivationFunctionType.Tanh
        doctored = []
        seen_pref = False
        for name, fns in tables:
            if name == pref:
                seen_pref = True
                doctored.append((name, fns))
            elif not seen_pref:
                doctored.append((name, {TANH} & fns))
            else:
                doctored.append((name, fns))
        _bass_rust.insert_act_table_loads(self, doctored)

P = 128
D = 256
H = 8
DH = 32
SCALE = 1.0 / float(np.sqrt(DH))
N_CORES = 8
NEG = -1e9

F32 = mybir.dt.float32
BF16 = mybir.dt.bfloat16

# activation dtype switch ("f32" or "bf16")
DT_ACT_NAME = "bf16"


def _dt_act():
    return BF16 if DT_ACT_NAME == "bf16" else F32


def _np_act():
    return ml_dtypes.bfloat16 if DT_ACT_NAME == "bf16" else np.float32


# ---------------------------------------------------------------------------
# weight packing layout (shared between host packer and device program)
# ---------------------------------------------------------------------------
# W_all [128, n_wcols] (dt_act): matmul weight chunks, 128 cols each.
#   chunk_col(base, k, j) = base + k*(2*128) + j*128   (k-outer, j-inner)
#   lin_W  at base 0                      (4 chunks)
#   Wq[i]  at 512 + i*2048 + 0
#   Wk[i]  at 512 + i*2048 + 512
#   Wv[i]  at 512 + i*2048 + 1024
#   Wo[i]  at 512 + i*2048 + 1536
N_WCOLS = 512 + 2 * 2048

LIN_BASE = 0


def w_base(i, which):
    return 512 + i * 2048 + {"q": 0, "k": 512, "v": 1024, "o": 1536}[which]


# C_all [128, n_ccols] f32: per-feature columns (partition = feature within
# d-tile j). col index helpers:
#   0,1   lin_b (j=0,1)
#   2,3   lin_g
#   4,5   lin_beta
#   6+i*12 + [0,1]=bq, [2,3]=bk, [4,5]=bv, [6,7]=bo, [8,9]=ln_g, [10,11]=ln_b
#   30..30+T  maskbias columns (per key-tile)
def c_lin(which, j):
    return {"b": 0, "g": 2, "beta": 4}[which] + j


def c_blk(i, which, j):
    return 6 + i * 12 + {"q": 0, "k": 2, "v": 4, "o": 6, "g": 8, "beta": 10}[which] + j


C_FIXED = 30


# ---------------------------------------------------------------------------
# device program
# ---------------------------------------------------------------------------
def build_program(slot_ts, trivial_ln):
    """slot_ts: tuple of per-slot tile counts (shared across cores).
    trivial_ln: all LN gains are 1 and shifts 0 (skip gamma/beta application)
    """
    dt = _dt_act()
    T = int(sum(slot_ts))
    NC = T * P  # padded node count per core
    CHW = 512  # chunk width for the n dimension
    NCH = [(c0, min(CHW, NC - c0)) for c0 in range(0, NC, CHW)]  # n chunks

    nc = _Bacc()
    featT = nc.declare_dram_parameter("featT", [P, 2, NC], dt, isOutput=False)
    wall = nc.declare_dram_parameter("wall", [P, N_WCOLS], dt, isOutput=False)
    cons = nc.declare_dram_parameter("cons", [P, C_FIXED + T], F32, isOutput=False)
    outT = nc.declare_dram_parameter("outT", [P, 2, NC], F32, isOutput=True)

    with tile.TileContext(nc) as tc:
        with (
            tc.tile_pool(name="wp", bufs=1) as wp,
            tc.tile_pool(name="xp", bufs=1) as xp,
            tc.tile_pool(name="pp", bufs=max(4, 2 * max(slot_ts))) as pp,
            tc.tile_pool(name="rows", bufs=2) as rows,
            tc.tile_pool(name="psA", bufs=2, space="PSUM") as psA,
            tc.tile_pool(name="psS", bufs=2, space="PSUM") as psS,
            tc.tile_pool(name="psB", bufs=1, space="PSUM") as psB,
        ):
            # separate weight tiles per stage so consumers only wait on
            # their own DMA (tile-granular deps)
            w_lin = wp.tile([P, 512], dt, tag="w_lin")
            w_blk = [wp.tile([P, 2048], dt, tag=f"w_blk{i}", name=f"w_blk{i}")
                     for i in range(2)]
            c_sb = wp.tile([P, C_FIXED + T], F32, tag="c")
            nc.sync.dma_start(c_sb[:], cons[:])
            nc.sync.dma_start(w_lin[:], wall[:, 0:512])
            # big weight loads go on other queues so x0/w_lin aren't stuck
            # behind them and block0 can start immediately
            nc.scalar.dma_start(w_blk[0][:], wall[:, 512:2560])
            nc.gpsimd.dma_start(w_blk[1][:], wall[:, 2560:4608])

            def w_tile_of(base):
                if base < 512:
                    return w_lin, base
                i = (base - 512) // 2048
                return w_blk[i], (base - 512) % 2048

            x0 = [xp.tile([P, NC], dt, tag=f"x0_{k}", name=f"x0_{k}") for k in range(2)]
            nc.sync.dma_start(x0[0][:], featT[:, 0, :])
            nc.sync.dma_start(x0[1][:], featT[:, 1, :])

            # constants
            ones32 = wp.tile([P, 32], BF16, tag="ones32")
            nc.vector.memset(ones32, 1.0)
            c256 = wp.tile([P, 1], dt, tag="c256")
            nc.vector.memset(c256, 1.0 / 256.0)
            ones_row = wp.tile([1, P], dt, tag="ones_row")
            nc.vector.memset(ones_row, 1.0)
            eps_row = wp.tile([1, 1], F32, tag="eps_row")
            nc.vector.memset(eps_row, 1e-5)

            def wcol(base, k, j, width=P):
                wt, rel = w_tile_of(base)
                c0 = rel + k * 256 + j * 128
                return wt[:, c0 : c0 + width]

            def ccol(idx):
                return c_sb[:, idx : idx + 1]

            def r32(ap):
                # float32r: same bits as f32, single-pass PE mode (vs the
                # 2-pass LOW_HIGH fp32 lowering)
                return ap.bitcast(mybir.dt.float32r) if ap.dtype == F32 else ap

            def linearT(src, base, bias_idx, func, out_dt, eng="act"):
                """out[j][d,n] = func(sum_k W[k,j].T @ src[k] + bias_j)"""
                out = [xp.tile([P, NC], out_dt, tag=f"lt{base}_{j}", name=f"lt{base}_{j}") for j in range(2)]
                for j in range(2):
                    for c0, cw in NCH:
                        ps = psA.tile([P, cw], F32, tag="work")
                        for k in range(2):
                            nc.tensor.matmul(
                                ps,
                                r32(wcol(base, k, j)),
                                r32(src[k][:, c0 : c0 + cw]),
                                start=(k == 0),
                                stop=(k == 1),
                            )
                        if eng == "dve":
                            nc.vector.tensor_scalar_add(
                                out[j][:, c0 : c0 + cw], ps, ccol(bias_idx + j)
                            )
                        else:
                            nc.scalar.activation(
                                out[j][:, c0 : c0 + cw], ps, func,
                                bias=ccol(bias_idx + j),
                            )
                return out

            def layernormT(y, gcol, bcol, out_dt, trivial):
                """LayerNorm over feature dim (partitions, 2 tiles of 128)."""
                out = [xp.tile([P, NC], out_dt, tag=f"ln_{k}", name=f"ln_{k}") for k in range(2)]
                # squared input (same dtype for matmul rhs)
                sq = [xp.tile([P, NC], dt, tag=f"sq_{k}", name=f"sq_{k}") for k in range(2)]
                for k in range(2):
                    nc.vector.tensor_mul(sq[k], y[k], y[k])
                for c0, cw in NCH:
                    stats = psA.tile([33, cw], F32, tag="work")
                    for k in range(2):
                        nc.tensor.matmul(
                            stats[0:1, :],
                            r32(c256),
                            r32(y[k][:, c0 : c0 + cw]),
                            start=(k == 0),
                            stop=(k == 1),
                        )
                    for k in range(2):
                        nc.tensor.matmul(
                            stats[32:33, :],
                            r32(c256),
                            r32(sq[k][:, c0 : c0 + cw]),
                            start=(k == 0),
                            stop=(k == 1),
                            tile_position=(0, 32),
                        )
                    # rows: var = E[y^2] - mean^2 ; rstd = 1/sqrt(var+eps)
                    m2 = rows.tile([1, cw], F32, tag="m2")
                    nc.scalar.activation(
                        m2, stats[0:1, :], mybir.ActivationFunctionType.Square
                    )
                    var = rows.tile([1, cw], F32, tag="var")
                    nc.vector.scalar_tensor_tensor(
                        var,
                        m2,
                        -1.0,
                        stats[32:33, :],
                        op0=mybir.AluOpType.mult,
                        op1=mybir.AluOpType.add,
                    )
                    # rstd = exp(-0.5*ln(var+eps)) — Ln/Exp share one ACT
                    # table set; Sqrt would force a ~2.7us table switch
                    lnv = rows.tile([1, cw], F32, tag="lnv")
                    nc.scalar.activation(
                        lnv, var, mybir.ActivationFunctionType.Ln, bias=eps_row[:]
                    )
                    rstd = rows.tile([1, cw], dt, tag="rstd")
                    nc.scalar.activation(
                        rstd, lnv, mybir.ActivationFunctionType.Exp, scale=-0.5
                    )
                    ms = rows.tile([1, cw], dt, tag="ms")
                    nc.vector.tensor_mul(ms, stats[0:1, :], rstd)
                    # broadcast rows to 128 partitions via K=1 matmuls
                    sb = psA.tile([P, cw], F32, tag="work")
                    nc.tensor.matmul(sb, r32(ones_row), r32(rstd), start=True, stop=True)
                    msb = psA.tile([P, cw], F32, tag="work")
                    nc.tensor.matmul(msb, r32(ones_row), r32(ms), start=True, stop=True)
                    for k in range(2):
                        t1 = xp.tile([P, cw], F32, tag="ln_t1")
                        nc.vector.tensor_mul(t1, y[k][:, c0 : c0 + cw], sb)
                        dst = out[k][:, c0 : c0 + cw]
                        if trivial:
                            nc.vector.tensor_sub(dst, t1, msb)
                        else:
                            t2 = xp.tile([P, cw], F32, tag="ln_t2")
                            nc.vector.tensor_sub(t2, t1, msb)
                            nc.vector.tensor_scalar(
                                dst,
                                t2,
                                ccol(gcol + k),
                                ccol(bcol + k),
                                op0=mybir.AluOpType.mult,
                                op1=mybir.AluOpType.add,
                            )
                return out

            def attention_block(i, hT):
                """one MHA block: returns new xT (list of 2 tiles)."""
                qT = linearT(
                    hT, w_base(i, "q"), c_blk(i, "q", 0),
                    mybir.ActivationFunctionType.Identity, BF16,
                )
                kT = linearT(
                    hT, w_base(i, "k"), c_blk(i, "k", 0),
                    mybir.ActivationFunctionType.Identity, BF16, eng="dve",
                )
                # matmul operands must start at partition 0 on this stack, so
                # shift each head's 32 rows down via SBUF->SBUF DMA
                q_h = xp.tile([32, H, NC], BF16, tag="q_h", name="q_h")
                k_h = xp.tile([32, H, NC], BF16, tag="k_h", name="k_h")
                # spread the 16 shift-DMAs across engine queues so they run
                # in parallel instead of serializing on the sync queue
                engs = [nc.sync, nc.gpsimd, nc.scalar]
                for h in range(H):
                    b, hh = divmod(h, 4)
                    engs[h % 3].dma_start(
                        q_h[:, h, :], qT[b][32 * hh : 32 * hh + 32, :]
                    )
                    engs[(h + 1) % 3].dma_start(
                        k_h[:, h, :], kT[b][32 * hh : 32 * hh + 32, :]
                    )
                # v in node layout: v[n_tile, d] = hT_chunk.T @ Wv_chunk
                v_sb = xp.tile([P, T, 256], BF16, tag="v_all")
                for t in range(T):
                    vp = psA.tile([P, 256], F32, tag="work")
                    for k in range(2):
                        nc.tensor.matmul(
                            vp,
                            r32(hT[k][:, t * P : (t + 1) * P]),
                            r32(wcol(w_base(i, "v"), k, 0, width=256)),
                            start=(k == 0),
                            stop=(k == 1),
                        )
                    nc.scalar.activation(
                        v_sb[:, t, :], vp, mybir.ActivationFunctionType.Copy
                    )
                # attention per slot
                o_sb = [xp.tile([P, NC], dt, tag=f"o_{k}", name=f"o_{k}") for k in range(2)]
                # oT/DN accumulate banks per (dtile, nchunk)
                all_unit = all(t == 1 for t in slot_ts)
                for c0, cw in NCH:
                    oT_ps = [psB.tile([P, cw], F32, tag=f"oT{k}", name=f"oT{k}") for k in range(2)]
                    dn_ps = None
                    if not all_unit:
                        dn_ps = [psA.tile([P, cw], F32, tag="work", name=f"dn{k}") for k in range(2)]
                    # pT for the whole chunk lives in one tile so exp is one
                    # op per (slot, key-tile) and the denominator one matmul
                    # per (bank, head)
                    ctiles = cw // P
                    pT_all = pp.tile([P, ctiles, 2, 4, P], BF16, tag="pTall",
                                     name="pT_all")
                    t_off = 0  # key-tile offset (global)
                    q_off = 0
                    for s, ts_s in enumerate(slot_ts):
                        for qc0 in range(q_off, q_off + ts_s * P, P):
                            if qc0 < c0 or qc0 >= c0 + cw:
                                continue
                            qrel = qc0 - c0
                            qi = qrel // P
                            # scores + exp for all key tiles of this slot
                            pT = {}
                            for kt in range(ts_s):
                                ktg = t_off + kt
                                stp = psS.tile([P, 2, 4, P], F32, tag="stp",
                                               name="stp")
                                for b in range(2):
                                    for hh in range(4):
                                        nc.tensor.matmul(
                                            stp[:, b, hh, :],
                                            k_h[:, 4 * b + hh,
                                                ktg * P : (ktg + 1) * P],
                                            q_h[:, 4 * b + hh, qc0 : qc0 + P],
                                            start=True,
                                            stop=True,
                                        )
                                if all_unit and ts_s == 1:
                                    p_t = pT_all[:, qi, :, :, :]
                                else:
                                    p_t = pp.tile([P, 2, 4, P], BF16, tag="pT",
                                                  name="p_t")
                                nc.scalar.activation(
                                    p_t,
                                    stp,
                                    mybir.ActivationFunctionType.Exp,
                                    scale=SCALE,
                                    bias=ccol(C_FIXED + ktg),
                                )
                                pT[kt] = p_t
                            # PV accumulation, one closed psum group per
                            # (bank, head) at a time
                            for b in range(2):
                                for hh in range(4):
                                    for kt in range(ts_s):
                                        nc.tensor.matmul(
                                            oT_ps[b][32 * hh : 32 * hh + 32,
                                                     qrel : qrel + P],
                                            v_sb[:, t_off + kt,
                                                 (4 * b + hh) * 32 : (4 * b + hh) * 32 + 32],
                                            pT[kt][:, b, hh, :],
                                            start=(kt == 0),
                                            stop=(kt == ts_s - 1),
                                            tile_position=(0, 32 * hh),
                                        )
                                    if not all_unit:
                                        for kt in range(ts_s):
                                            nc.tensor.matmul(
                                                dn_ps[b][32 * hh : 32 * hh + 32,
                                                         qrel : qrel + P],
                                                ones32,
                                                pT[kt][:, b, hh, :],
                                                start=(kt == 0),
                                                stop=(kt == ts_s - 1),
                                                tile_position=(0, 32 * hh),
                                            )
                        t_off += ts_s
                        q_off += ts_s * P
                    if all_unit:
                        # merged denominators: each slot's keys live on the
                        # partition axis of its own pT block, so one matmul
                        # per (bank, head) column-sums the whole chunk
                        dn_ps = [psA.tile([P, cw], F32, tag="work", name=f"dn{k}")
                                 for k in range(2)]
                        for b in range(2):
                            for hh in range(4):
                                nc.tensor.matmul(
                                    dn_ps[b][32 * hh : 32 * hh + 32, :],
                                    ones32,
                                    pT_all[:, :, b, hh, :],
                                    start=True,
                                    stop=True,
                                    tile_position=(0, 32 * hh),
                                )
                    # normalize: o = oT * (1/dn), fused with PSUM->SBUF copy
                    for k in range(2):
                        r_sb = pp.tile([P, cw], F32, tag="r")
                        nc.vector.reciprocal_approx_fast(out=r_sb, in_=dn_ps[k])
                        nc.vector.tensor_mul(o_sb[k][:, c0 : c0 + cw], oT_ps[k], r_sb)
                # output projection + residual + LN
                y = [xp.tile([P, NC], dt, tag=f"y_{k}", name=f"y_{k}") for k in range(2)]
                for j in range(2):
                    for c0, cw in NCH:
                        zp = psA.tile([P, cw], F32, tag="work")
                        for k in range(2):
                            nc.tensor.matmul(
                                zp,
                                r32(wcol(w_base(i, "o"), k, j)),
                                r32(o_sb[k][:, c0 : c0 + cw]),
                                start=(k == 0),
                                stop=(k == 1),
                            )
                        nc.vector.scalar_tensor_tensor(
                            y[j][:, c0 : c0 + cw],
                            zp,
                            ccol(c_blk(i, "o", j)),
                            hT[j][:, c0 : c0 + cw],
                            op0=mybir.AluOpType.add,
                            op1=mybir.AluOpType.add,
                        )
                out_dt = F32 if i == 1 else dt
                return layernormT(
                    y, c_blk(i, "g", 0), c_blk(i, "beta", 0), out_dt, trivial_ln
                )

            # block 0 pre-layer: LN(tanh(x @ lin_W + lin_b)) * g + beta
            t0 = linearT(
                x0, LIN_BASE, c_lin("b", 0), mybir.ActivationFunctionType.Tanh, dt
            )
            h0 = layernormT(t0, c_lin("g", 0), c_lin("beta", 0), dt, trivial_ln)
            x1 = attention_block(0, h0)
            x2 = attention_block(1, x1)
            nc.sync.dma_start(outT[:, 0, :], x2[0][:])
            nc.sync.dma_start(outT[:, 1, :], x2[1][:])

    nc.finalize()
    return nc


# ---------------------------------------------------------------------------
# host side
# ---------------------------------------------------------------------------
_prog_cache = {}
_last_results = None


def _get_program(slot_ts, trivial_ln):
    key = (tuple(slot_ts), trivial_ln, DT_ACT_NAME)
    if key not in _prog_cache:
        _prog_cache[key] = build_program(tuple(slot_ts), trivial_ln)
    return _prog_cache[key]


def _segments(ordering):
    """contiguous runs of equal values in sorted ordering -> (start, len)."""
    n = ordering.shape[0]
    change = np.nonzero(np.diff(ordering))[0] + 1
    starts = np.concatenate([[0], change])
    lens = np.diff(np.concatenate([starts, [n]]))
    return list(zip(starts.tolist(), lens.tolist()))


def kernel(
    feat, ordering, lin_W, lin_b, lin_g, lin_beta,
    Wq, Wk, Wv, bq, bk, bv, Wo, bo, ln_g, ln_b,
):
    feat = np.asarray(feat, np.float32)
    ordering = np.asarray(ordering)
    N = feat.shape[0]
    np_act = _np_act()

    perm = np.argsort(ordering, kind="stable")
    segs = _segments(np.asarray(ordering)[perm])

    # deal segments (sorted by length desc) snake-wise to cores
    order = sorted(range(len(segs)), key=lambda i: -segs[i][1])
    core_slots = [[] for _ in range(N_CORES)]
    for r, si in enumerate(order):
        c = r % (2 * N_CORES)
        c = c if c < N_CORES else 2 * N_CORES - 1 - c
        core_slots[c].append(si)
    S = max(len(cs) for cs in core_slots)
    # per-slot tile count = max over cores of the slot's segment size
    slot_ts = []
    for k in range(S):
        mx = 1
        for c in range(N_CORES):
            if k < len(core_slots[c]):
                mx = max(mx, (segs[core_slots[c][k]][1] + P - 1) // P)
        slot_ts.append(mx)
    T = sum(slot_ts)
    NC = T * P

    trivial_ln = bool(
        np.all(np.asarray(ln_g) == 1) and np.all(np.asarray(ln_b) == 0)
        and np.all(np.asarray(lin_g) == 1) and np.all(np.asarray(lin_beta) == 0)
    )
    # softmax rows sum to 1, so attn @ (v + 1*bv^T) = attn@v + 1*bv^T; the
    # per-block value bias folds exactly into the output-projection bias:
    # bo_eff = bo + bv @ Wo
    bo = np.asarray(bo, np.float32) + np.einsum(
        "id,idj->ij", np.asarray(bv, np.float32), np.asarray(Wo, np.float32)
    )

    nc = _get_program(slot_ts, trivial_ln)

    # ---- pack weights ----
    wallp = np.zeros((P, N_WCOLS), np.float32)

    def put_w(base, W):
        W = np.asarray(W, np.float32)
        for k in range(2):
            for j in range(2):
                c0 = base + k * 256 + j * 128
                wallp[:, c0 : c0 + 128] = W[k * 128 : (k + 1) * 128,
                                            j * 128 : (j + 1) * 128]

    put_w(LIN_BASE, lin_W)
    for i in range(2):
        put_w(w_base(i, "q"), np.asarray(Wq)[i])
        put_w(w_base(i, "k"), np.asarray(Wk)[i])
        put_w(w_base(i, "v"), np.asarray(Wv)[i])
        put_w(w_base(i, "o"), np.asarray(Wo)[i])
    wallp = wallp.astype(np_act)

    consp = np.zeros((P, C_FIXED + T), np.float32)

    def put_c(idx, vec):
        vec = np.asarray(vec, np.float32)
        consp[:, idx] = vec[:128]
        consp[:, idx + 1] = vec[128:]

    put_c(c_lin("b", 0), lin_b)
    put_c(c_lin("g", 0), lin_g)
    put_c(c_lin("beta", 0), lin_beta)
    for i in range(2):
        put_c(c_blk(i, "q", 0), np.asarray(bq)[i])
        put_c(c_blk(i, "k", 0), np.asarray(bk)[i])
        put_c(c_blk(i, "v", 0), np.asarray(bv)[i] * 0)
        put_c(c_blk(i, "o", 0), np.asarray(bo)[i])
        put_c(c_blk(i, "g", 0), np.asarray(ln_g)[i])
        put_c(c_blk(i, "beta", 0), np.asarray(ln_b)[i])

    # ---- per-core data ----
    feat_sorted = feat[perm]
    in_maps = []
    core_meta = []  # (slot k, seg_start, seg_len, node_offset in padded layout)
    for c in range(N_CORES):
        fT = np.zeros((NC, 256), np.float32)
        mb = np.full((NC,), NEG, np.float32)
        meta = []
        off = 0
        for k in range(S):
            if k < len(core_slots[c]):
                st, ln = segs[core_slots[c][k]]
                fT[off : off + ln] = feat_sorted[st : st + ln]
                mb[off : off + ln] = 0.0
                meta.append((st, ln, off))
            off += slot_ts[k] * P
        cons_c = consp.copy()
        cons_c[:, C_FIXED : C_FIXED + T] = mb.reshape(T, P).T
        featT_c = np.ascontiguousarray(
            fT.T.reshape(2, P, NC).transpose(1, 0, 2)
        ).astype(np_act)
        in_maps.append({"featT": featT_c, "wall": wallp, "cons": cons_c})
        core_meta.append(meta)

    res = run_bass_kernel_spmd(nc, in_maps, list(range(N_CORES)))
    global _last_results
    _last_results = res

    out = np.empty((N, 256), np.float32)
    for c in range(N_CORES):
        oT = np.asarray(res.results[c]["outT"], np.float32)  # [128, 2, NC]
        o_nodes = oT.transpose(1, 0, 2).reshape(256, NC).T  # [NC, 256]
        for st, ln, off in core_meta[c]:
            out[perm[st : st + ln]] = o_nodes[off : off + ln]
    return out
